# revision 67
# baseline (speedup 1.0000x reference)
"""BinaryLinear Trainium2 kernel: Y = X @ binarize(W).T + bias.

Shapes (hardcoded per the problem spec):
  X: [8192, 4096] f32, W: [4096, 4096] f32, bias: [4096] f32 -> Y: [8192, 4096] f32

Strategy: data-parallel over tokens across 8 NeuronCores (1024 tokens/core),
weight replicated; no collectives. The shipped mode is `fp8s`:

  Y = e4m3(bf16(X)) @ (binarize(W) - 1/2).T + 1/2*rowsum(bf16(X)) + bias

fp8 e4m3 DoubleRow matmuls run at 2x the bf16/fp32r MAC rate (measured 1.0
cycle per output row with K=256 per instruction vs K=128 at full rate). The
binary weights are exact in fp8, so only X's e4m3 rounding contributes error;
rewriting the binary mask as (Wb - 1/2) + 1/2*ones cancels the mask-mean of
that error (sqrt(2) reduction): measured rel err 1.92e-2 vs the 2e-2 gate,
bit-identical to the host-side simulation of the same arithmetic.

Per core: X^T ships bf16 pre-tiled (ACT-ring DMA), is cast to resident fp8 on
DVE, and its token row-sums accumulate in 2 spare PSUM banks via bf16
ones-matmuls, all interleaved with the first out-block's 6 leading m-chains so
the PE starts ~12us in and never idles. W^T ships as the f32 MSB byte (exact
sign info at 1 byte/weight), streams one out-block ahead on the Sync ring, and
binarizes to {-1/2,+1/2} fp8 in one 2-op DVE tensor_scalar per slab. Each
out-block runs 8 m-chains of 16 DoubleRow matmuls into rotating PSUM banks;
drains are a single fused scalar_tensor_tensor (psum + S/2 + bias) on DVE.
Out-block 0 drains psum+bias immediately and folds S in afterwards so the
S-transpose chain never stalls PSUM recycling.

Measured: 506us (f32r baseline) -> 257us, rel err 1.921982e-2.

Compute modes (env TRNKERNEL_MODE):
  fp8s   (default): the design above
  f32r   : fp32r matmuls — full-rate reduced-precision fp32 (rel err ~1e-4)
  bf16   : single-pass bf16 (X rounded to bf16)
  bf16x2 : X split into hi+lo bf16, two accumulating passes (near-fp32 exact)
  fp8dr  : single-pass fp8 e4m3 DoubleRow, no correction (rel err 2.4e-2)
  fp8dr2 : X split into hi+lo fp8 e4m3, two DoubleRow passes (rel err 7e-4)
"""
import os
import sys

import numpy as np

sys.path.insert(0, "/opt/trn_rl_repo")

import concourse.bacc as bacc
import concourse.mybir as mybir
import concourse.tile as tile
from concourse.bass_utils import run_bass_kernel_spmd

N_TOKENS = 8192
IN_F = 4096
OUT_F = 4096
N_CORES = 8
TOK_C = N_TOKENS // N_CORES  # 1024 tokens per core

P = 128
K_TILES = IN_F // P          # 32
KG = 4                       # k-tiles per W DMA/binarize group
K_GROUPS = K_TILES // KG     # 8
M_TILES = TOK_C // P         # 8
OB = 512                     # out-features per block (one PSUM bank)
O_BLOCKS = OUT_F // OB       # 8
XKG = 2                      # k-tiles per X-load DMA (1 MiB)

_MODE = os.environ.get("TRNKERNEL_MODE", "fp8s")
_TRACE = os.environ.get("TRNKERNEL_TRACE", "0") == "1"

_CACHED = {}


def _install_ntff_shim():
    """Register the NTFF profile hook so trace=True yields exec_time_ns."""
    import types

    try:
        import antenv  # noqa: F401
        from trn_agent_boot.trn_boot import _ntff_profile_via_ctypes
        import concourse.bass_utils as bu

        hook = _ntff_profile_via_ctypes("/opt/axon/libaxon_pjrt.so")
        mod = types.ModuleType("antenv.axon_hooks")
        mod.get_axon_ntff_profile_hook = lambda: hook
        mod.set_axon_ntff_profile_hook = lambda h: None
        sys.modules["antenv.axon_hooks"] = mod
        bu.upload_artifacts = lambda tmpdir: tmpdir  # no artifact store here
    except Exception:
        pass


def build(mode: str):
    assert mode in ("f32r", "bf16", "bf16x2", "fp8dr", "fp8dr2")
    fp8 = mode.startswith("fp8")
    if mode == "f32r":
        mm_dt = mybir.dt.float32r
    elif fp8:
        mm_dt = mybir.dt.float8e4
    else:
        mm_dt = mybir.dt.bfloat16

    nc = bacc.Bacc(None)
    xt = nc.declare_dram_parameter("xt", [IN_F, TOK_C], mybir.dt.float32, isOutput=False)
    # W^T ships as bf16: only sign(w) is consumed (binarize on device), and
    # bf16 preserves the sign of every representable nonzero f32 from this
    # input scale; halving W bytes removes the DMA bottleneck of the first
    # out-block (X + W streams exceed the 358 GB/s HBM limit otherwise).
    wt = nc.declare_dram_parameter("wt", [IN_F, OUT_F], mybir.dt.bfloat16, isOutput=False)
    bias = nc.declare_dram_parameter("bias", [OUT_F], mybir.dt.float32, isOutput=False)
    y = nc.declare_dram_parameter("y", [TOK_C, OUT_F], mybir.dt.float32, isOutput=True)

    # DRAM-side tiled views: partition dim = contraction (in-features)
    xt_v = xt.rearrange("(kt p) t -> p kt t", p=P)      # [128, 32, 1024]
    wt_v = wt.rearrange("(kt p) o -> p kt o", p=P)      # [128, 32, 4096]
    y_v = y.rearrange("(mt p) o -> p mt o", p=P)        # [128, 8, 4096]

    n_x = 2 if mode in ("bf16x2", "fp8dr2") else 1
    two_pass = n_x == 2

    with tile.TileContext(nc) as tc:
        with (
            tc.tile_pool(name="xres", bufs=1) as xres_pool,
            tc.tile_pool(name="xstage", bufs=2) as xstage_pool,
            tc.tile_pool(name="wstage", bufs=3) as wstage_pool,
            tc.tile_pool(name="wb", bufs=3) as wb_pool,
            tc.tile_pool(name="biasp", bufs=1) as bias_pool,
            tc.tile_pool(name="osb", bufs=4) as osb_pool,
            tc.tile_pool(name="psum", bufs=1, space="PSUM") as psum_pool,
        ):
            xr = [
                xres_pool.tile([P, K_TILES, TOK_C], mm_dt, tag=f"xr{i}", name=f"xr{i}")
                for i in range(n_x)
            ]

            def load_x_chunk(kk):
                """DMA one [128, XKG, 1024] X^T chunk and round into xr (ACT)."""
                xs = xstage_pool.tile([P, XKG, TOK_C], mybir.dt.float32, name="xs")
                nc.sync.dma_start(out=xs[:], in_=xt_v[:, kk * XKG:(kk + 1) * XKG, :])
                sl = slice(kk * XKG, (kk + 1) * XKG)
                nc.vector.tensor_scalar(
                    out=xr[0][:, sl, :], in0=xs[:], scalar1=0.0, scalar2=None,
                    op0=mybir.AluOpType.add,
                )
                if two_pass:
                    nc.vector.tensor_sub(out=xr[1][:, sl, :], in0=xs[:], in1=xr[0][:, sl, :])

            for ob in range(O_BLOCKS):
                osl = slice(ob * OB, (ob + 1) * OB)

                psums = [psum_pool.tile([P, OB], mybir.dt.float32, name=f"ps{_m}") for _m in range(M_TILES)]

                for kg in range(K_GROUPS):
                    ckg = KG // XKG
                    if ob == 0:
                        # interleave X residency build into the first out-block;
                        # first chunk ahead of the W slab so MM k=0 unblocks early
                        load_x_chunk(kg * ckg)
                    ws = wstage_pool.tile([P, KG, OB], mybir.dt.bfloat16, name="ws")
                    nc.sync.dma_start(out=ws[:], in_=wt_v[:, kg * KG:(kg + 1) * KG, osl])
                    if ob == 0:
                        for j in range(1, ckg):
                            load_x_chunk(kg * ckg + j)
                    wb = wb_pool.tile([P, KG, OB], mm_dt, name="wb")
                    nc.vector.tensor_scalar(
                        out=wb[:], in0=ws[:], scalar1=0.0, scalar2=None,
                        op0=mybir.AluOpType.is_gt,
                    )
                    if fp8:
                        # DoubleRow: each matmul contracts K=256 (2 k-tiles
                        # as dim1 of both operands) at double throughput
                        kt2_last = K_TILES // 2 - 1
                        for ks2 in range(KG // 2):
                            kt2 = kg * (KG // 2) + ks2
                            ksl = slice(2 * ks2, 2 * ks2 + 2)
                            for m in range(M_TILES):
                                nc.tensor.matmul(
                                    out=psums[m][:],
                                    lhsT=xr[0][:, 2 * kt2:2 * kt2 + 2, m * P:(m + 1) * P],
                                    rhs=wb[:, ksl, :],
                                    start=(kt2 == 0),
                                    stop=(kt2 == kt2_last) and not two_pass,
                                    perf_mode=mybir.MatmulPerfMode.DoubleRow,
                                )
                                if two_pass:
                                    nc.tensor.matmul(
                                        out=psums[m][:],
                                        lhsT=xr[1][:, 2 * kt2:2 * kt2 + 2, m * P:(m + 1) * P],
                                        rhs=wb[:, ksl, :],
                                        start=False,
                                        stop=(kt2 == kt2_last),
                                        perf_mode=mybir.MatmulPerfMode.DoubleRow,
                                    )
                    else:
                        for ks in range(KG):
                            k = kg * KG + ks
                            for m in range(M_TILES):
                                nc.tensor.matmul(
                                    out=psums[m][:],
                                    lhsT=xr[0][:, k, m * P:(m + 1) * P],
                                    rhs=wb[:, ks, :],
                                    start=(k == 0),
                                    stop=(k == K_TILES - 1) if not two_pass else False,
                                )
                                if two_pass:
                                    nc.tensor.matmul(
                                        out=psums[m][:],
                                        lhsT=xr[1][:, k, m * P:(m + 1) * P],
                                        rhs=wb[:, ks, :],
                                        start=False,
                                        stop=(k == K_TILES - 1),
                                    )

                # bias for this out-block, broadcast across partitions; emitted
                # after the k-loop so its DMA never delays the W stream (ACT
                # copy so the DVE bias-add waits on a single semaphore)
                bstage = bias_pool.tile([P, OB], mybir.dt.float32, tag="bstage", name="bstage")
                nc.sync.dma_start(out=bstage[:], in_=bias[None, osl].to_broadcast([P, OB]))
                bias_bc = bias_pool.tile([P, OB], mybir.dt.float32, tag="bbc", name="bias_bc")
                nc.scalar.copy(out=bias_bc[:], in_=bstage[:])

                # drain: psum -> sbuf (ACT), + bias (DVE), -> DRAM
                for m in range(M_TILES):
                    o_sb = osb_pool.tile([P, OB], mybir.dt.float32, name="o_sb")
                    nc.scalar.copy(out=o_sb[:], in_=psums[m][:])
                    nc.vector.tensor_add(out=o_sb[:], in0=o_sb[:], in1=bias_bc[:])
                    nc.sync.dma_start(out=y_v[:, m, osl], in_=o_sb[:])

    nc.compile()
    return nc


def build_fp8s():
    """fp8 e4m3 DoubleRow single-pass + rank-1 correction (S-form).

    Y = Xq @ (Wb - 1/2).T + 1/2*rowsum(Xbf16) + bias, where Xq = e4m3(bf16(X)),
    Wb = (W > 0). The +-1/2 weights and the row-sum term cancel the mean of the
    e4m3 quantization error over the binary mask (sqrt(2) error reduction vs
    plain fp8; measured rel err 1.92e-2 vs the 2e-2 gate on these inputs).

    Inputs ship as X^T bf16 (halves X DMA) and W^T MSB bytes (sign+exponent
    byte of each f32; w>0 <=> int8 msb > 0 for all nonzero-magnitude w >=
    2^-125, exact on this data). Per out-block, W binarizes to {-1/2,+1/2} fp8
    in one 2-op tensor_scalar. Token row-sums S accumulate in 2 spare PSUM
    banks via bf16 ones-matmuls during X staging; a 4 KiB DMA transposes S to
    per-partition layout. Drain = one fused scalar_tensor_tensor:
    (psum + S/2) + bias -> SBUF -> DMA.
    """
    fp8 = mybir.dt.float8e4
    DR = mybir.MatmulPerfMode.DoubleRow
    XKG = 2                      # k-tiles per X chunk
    NCH = K_TILES // XKG         # 16 X chunks of 2 k-tiles (0.5 MiB each)
    PM = 6                       # m-chains interleaved into the prologue
    KT2 = K_TILES // 2           # 16 DoubleRow steps over K

    nc = bacc.Bacc(None)
    # Host pre-tiles inputs so every DMA lands contiguous per partition:
    # xt[c, p, j, t] = X^T chunk c (2 k-tiles), wt[ob, p, kt, o] = W^T msb.
    xt = nc.declare_dram_parameter("xt", [NCH, P, XKG, TOK_C], mybir.dt.bfloat16, isOutput=False)
    wt = nc.declare_dram_parameter("wt", [O_BLOCKS, P, K_TILES, OB], mybir.dt.int8, isOutput=False)
    bias = nc.declare_dram_parameter("bias", [OUT_F], mybir.dt.float32, isOutput=False)
    y = nc.declare_dram_parameter("y", [TOK_C, OUT_F], mybir.dt.float32, isOutput=True)

    y_v = y.rearrange("(mt p) o -> p mt o", p=P)        # [128, 8, 4096] f32

    with tile.TileContext(nc) as tc:
        with (
            tc.tile_pool(name="xres", bufs=1) as xres_pool,
            tc.tile_pool(name="xstage", bufs=5) as xstage_pool,
            tc.tile_pool(name="wstage", bufs=3) as ws_pool,
            tc.tile_pool(name="wb", bufs=2) as wb_pool,
            tc.tile_pool(name="small", bufs=1) as small_pool,
            tc.tile_pool(name="biasp", bufs=2) as bias_pool,
            tc.tile_pool(name="osb", bufs=10) as osb_pool,
            tc.tile_pool(name="psum", bufs=6, space="PSUM") as psum_pool,
            tc.tile_pool(name="psumS", bufs=1, space="PSUM") as psumS_pool,
        ):
            xr = xres_pool.tile([P, K_TILES, TOK_C], fp8, tag="xr", name="xr")
            ones_bf = small_pool.tile([P, 1], mybir.dt.bfloat16, tag="ones", name="ones")
            nc.any.memset(ones_bf[:], 1.0)

            psum_S = [
                psumS_pool.tile([P, OB], mybir.dt.float32, tag=f"psS{h}", name=f"psS{h}")
                for h in range(2)
            ]

            wbs = {}

            def emit_w_group(ob, kg):
                """DMA one W slab (4 k-tiles x 512 outs) and binarize to +-1/2.

                Binarize runs on the otherwise-idle GpSimd engine so DVE (X
                casts + drains) and the PE never wait on it."""
                ws = ws_pool.tile([P, KG, OB], mybir.dt.int8, name="ws")
                nc.sync.dma_start(out=ws[:], in_=wt[ob, :, kg * KG:(kg + 1) * KG, :])
                nc.vector.tensor_scalar(
                    out=wbs[ob][:, kg * KG:(kg + 1) * KG, :], in0=ws[:],
                    scalar1=0.0, scalar2=0.5,
                    op0=mybir.AluOpType.is_gt, op1=mybir.AluOpType.subtract,
                )

            def emit_w(ob):
                wbs[ob] = wb_pool.tile([P, K_TILES, OB], fp8, name="wb")
                for kg in range(K_GROUPS):
                    emit_w_group(ob, kg)

            def emit_bias(ob):
                b = bias_pool.tile([P, OB], mybir.dt.float32, name="bias_bc")
                nc.sync.dma_start(
                    out=b[:], in_=bias[None, ob * OB:(ob + 1) * OB].to_broadcast([P, OB])
                )
                return b

            def drain(ob, m, psm, bias_t, S_half):
                o_sb = osb_pool.tile([P, OB], mybir.dt.float32, name="o_sb")
                nc.vector.scalar_tensor_tensor(
                    out=o_sb[:], in0=psm[:], scalar=S_half[:, m:m + 1], in1=bias_t[:],
                    op0=mybir.AluOpType.add, op1=mybir.AluOpType.add,
                )
                nc.sync.dma_start(out=y_v[:, m, ob * OB:(ob + 1) * OB], in_=o_sb[:])

            def mm_step(ob, m, psm, kt2):
                nc.tensor.matmul(
                    out=psm[:],
                    lhsT=xr[:, 2 * kt2:2 * kt2 + 2, m * P:(m + 1) * P],
                    rhs=wbs[ob][:, 2 * kt2:2 * kt2 + 2, :],
                    start=(kt2 == 0), stop=(kt2 == KT2 - 1), perf_mode=DR,
                )

            def mm_chain(ob, m, psm):
                for kt2 in range(KT2):
                    mm_step(ob, m, psm, kt2)

            def mm_chain_pair(ob, m0, psA, m1, psB):
                # interleave two chains so consecutive matmuls alternate PSUM
                # banks (avoids any same-bank back-to-back accumulate bubble)
                for kt2 in range(KT2):
                    mm_step(ob, m0, psA, kt2)
                    mm_step(ob, m1, psB, kt2)

            # ---- prologue: out-block 0, X staging + S accumulation fused in;
            # W for ob0 AND ob1 stream in slab-interleaved so ob1 never waits
            wbs[0] = wb_pool.tile([P, K_TILES, OB], fp8, name="wb")
            wbs[1] = wb_pool.tile([P, K_TILES, OB], fp8, name="wb")
            ps0 = [psum_pool.tile([P, OB], mybir.dt.float32, name="ps") for _m in range(PM)]
            for c in range(NCH):
                xs = xstage_pool.tile([P, XKG, TOK_C], mybir.dt.bfloat16, name="xs")
                # X streams on the ACT hw-DGE ring, parallel to W on Sync's
                nc.scalar.dma_start(out=xs[:], in_=xt[c])
                nc.vector.tensor_scalar(
                    out=xr[:, XKG * c:XKG * (c + 1), :], in0=xs[:], scalar1=0.0,
                    scalar2=None, op0=mybir.AluOpType.add,
                )
                emit_w_group(c % 2, c // 2)
                for j in range(XKG):
                    for h in range(2):
                        nc.tensor.matmul(
                            out=psum_S[h][0:1, :],
                            lhsT=ones_bf[:, 0:1],
                            rhs=xs[:, j, h * 512:(h + 1) * 512],
                            start=(c == 0 and j == 0), stop=(c == NCH - 1 and j == XKG - 1),
                        )
                for kk in range(XKG // 2):
                    kt2 = c * (XKG // 2) + kk
                    for m in range(PM):
                        nc.tensor.matmul(
                            out=ps0[m][:],
                            lhsT=xr[:, 2 * kt2:2 * kt2 + 2, m * P:(m + 1) * P],
                            rhs=wbs[0][:, 2 * kt2:2 * kt2 + 2, :],
                            start=(kt2 == 0), stop=(kt2 == KT2 - 1), perf_mode=DR,
                        )

            # S: psum -> sbuf f32, transpose to per-partition cols, scale by 1/2
            S_sb = small_pool.tile([1, TOK_C], mybir.dt.float32, tag="S_sb", name="S_sb")
            nc.scalar.copy(out=S_sb[0:1, 0:512], in_=psum_S[0][0:1, :])
            nc.scalar.copy(out=S_sb[0:1, 512:1024], in_=psum_S[1][0:1, :])
            S_col = small_pool.tile([P, M_TILES], mybir.dt.float32, tag="S_col", name="S_col")
            for m in range(M_TILES):
                nc.sync.dma_start(
                    out=S_col[:, m:m + 1], in_=S_sb[0:1, m * P:(m + 1) * P]
                )
            S_half = small_pool.tile([P, M_TILES], mybir.dt.float32, tag="S_half", name="S_half")

            # ---- rest of out-block 0. Drains here must not wait on the
            # S_col transpose chain (it lands ~6us after the prologue), so
            # ob0 drains psum+bias immediately (freeing PSUM for ob1) and a
            # cheap second pass folds the S correction in before the Y DMA.
            bias0 = emit_bias(0)
            o_sb0 = []

            def drain0_part1(psm):
                o_sb = osb_pool.tile([P, OB], mybir.dt.float32, name="o_sb")
                nc.vector.tensor_add(out=o_sb[:], in0=psm[:], in1=bias0[:])
                o_sb0.append(o_sb)

            for m in range(PM):
                drain0_part1(ps0[m])
            psA = psum_pool.tile([P, OB], mybir.dt.float32, name="ps")
            psB = psum_pool.tile([P, OB], mybir.dt.float32, name="ps")
            mm_chain_pair(0, PM, psA, PM + 1, psB)
            drain0_part1(psA)
            drain0_part1(psB)
            # S_half emitted only now: it blocks the in-order DVE queue until
            # the S_col transpose lands, and nothing before this needs it
            nc.vector.tensor_scalar(
                out=S_half[:], in0=S_col[:], scalar1=0.5, scalar2=None,
                op0=mybir.AluOpType.mult,
            )
            for m in range(M_TILES):
                nc.vector.tensor_scalar(
                    out=o_sb0[m][:], in0=o_sb0[m][:], scalar1=S_half[:, m:m + 1],
                    scalar2=None, op0=mybir.AluOpType.add,
                )
                nc.sync.dma_start(out=y_v[:, m, 0:OB], in_=o_sb0[m][:])

            # ---- out-blocks 1..7
            for ob in range(1, O_BLOCKS):
                bias_t = emit_bias(ob)
                if ob + 1 < O_BLOCKS:
                    emit_w(ob + 1)
                for mp in range(M_TILES // 2):
                    psA = psum_pool.tile([P, OB], mybir.dt.float32, name="ps")
                    psB = psum_pool.tile([P, OB], mybir.dt.float32, name="ps")
                    mm_chain_pair(ob, 2 * mp, psA, 2 * mp + 1, psB)
                    drain(ob, 2 * mp, psA, bias_t, S_half)
                    drain(ob, 2 * mp + 1, psB, bias_t, S_half)

    nc.compile()
    return nc


def kernel(X: np.ndarray, weight: np.ndarray, bias: np.ndarray) -> np.ndarray:
    assert X.shape == (N_TOKENS, IN_F) and weight.shape == (OUT_F, IN_F)
    mode = _MODE

    if mode not in _CACHED:
        _CACHED[mode] = build_fp8s() if mode == "fp8s" else build(mode)
    nc = _CACHED[mode]

    if _TRACE:
        _install_ntff_shim()

    # Host-side layout prep (sharding + transposes + dtype casts; math is
    # on-device)
    import ml_dtypes
    bias_np = np.ascontiguousarray(bias.astype(np.float32, copy=False))
    if mode == "fp8s":
        # W ships as the MSB byte of each f32 (sign + top 7 exponent bits):
        # w > 0 <=> signed msb byte > 0 for every |w| >= 2^-125, so the
        # device-side is_gt binarize is exact. X ships bf16. Both are
        # pre-tiled so every DMA line is contiguous per SBUF partition:
        #   wt[ob, p, kt, o] = msb(W^T)[kt*128 + p, ob*512 + o]
        #   xt[c, p, j, t]   = bf16(X_shard^T)[(2c + j)*128 + p, t]
        w_c = np.ascontiguousarray(weight, dtype=np.float32)
        msb = w_c.view(np.uint8).reshape(OUT_F, IN_F, 4)[:, :, 3]   # [out, in]
        # [out, in] -> [ob, o, kt, p] -> transpose to [ob, p, kt, o]
        wt_np = np.ascontiguousarray(
            msb.reshape(8, 512, 32, P).transpose(0, 3, 2, 1)
        ).view(np.int8)
        x_cast = X.astype(ml_dtypes.bfloat16)
    else:
        wt_np = np.ascontiguousarray(weight.T).astype(ml_dtypes.bfloat16)
        x_cast = X.astype(np.float32, copy=False)
    in_maps = []
    for c in range(N_CORES):
        xs = x_cast[c * TOK_C:(c + 1) * TOK_C, :]
        xt_np = np.ascontiguousarray(xs.T)
        if mode == "fp8s":
            # [4096, 1024] -> [16 chunks, 2, 128, 1024] -> [16, 128, 2, 1024]
            xt_np = np.ascontiguousarray(
                xt_np.reshape(16, 2, P, TOK_C).transpose(0, 2, 1, 3)
            )
        in_maps.append({"xt": xt_np, "wt": wt_np, "bias": bias_np})

    res = run_bass_kernel_spmd(
        nc, in_maps, core_ids=list(range(N_CORES)), trace=_TRACE,
    )
    out = np.concatenate([res.results[c]["y"] for c in range(N_CORES)], axis=0)
    if _TRACE:
        kernel.last_exec_time_ns = res.exec_time_ns
        kernel.last_trace = res.instructions_and_trace
    return out.astype(np.float32, copy=False)

